# revision 13
# baseline (speedup 1.0000x reference)
"""Trainium2 kernel for the algo/task performance-scan problem.

Restructuring: the lax.scan's only cross-step dependency is through the 64
scalars sig[:, lx[l]] read each step.  That scalar chain (O(A*L + L^2) work)
is computed on the host in float64.  Given the per-step coefficients
c[a,l] = eff[a] + s[a,l]*boost[a], the full field is a banded matmul

    result[a, l, t] = sum_{j<=l} mem[a]^(l-j) * c[a,j] * row_j[t]

(mem ~ 0.5-0.72, so terms with l-j > ~64 are below fp32 noise), followed by
sig = tanh(result / (2*diff))  (identity: 2*sigmoid(x)-1 = tanh(x/2)).

Precision: R and G are split into bf16 hi+lo parts.  The hi term Rh@Gh is
a K=128-window matmul per 64-wide l-tile.  The two correction terms only
matter for l-j <= ~32, so per 32-wide l-chunk they are packed into ONE
K=128 matmul: partitions 0-63 carry Rl_win@Gh_win, partitions 64-127 carry
Rh_win@Gl_win, both accumulating into the same PSUM columns.  Field error
~2e-5; the fp16 output DMA rounding (~2.4e-4) dominates.

Per core (8 algos): 192 matmuls [K=128, M=128 t, N=512/256] at full bf16
PE speed (~27us), tanh on ACT with per-partition 1/(2*diff) scale (~33us),
fp16 output in [g, t, a, l] layout so every partition stores one 4KB
contiguous run (host permutes back).  Sharding: 8 algos per core.
"""

import sys

sys.path.insert(0, "/opt/trn_rl_repo")

import numpy as np

A, T, L = 64, 1024, 512
NCORES = 8
ACORE = A // NCORES          # 8 algos per core
LT = 64                      # main l-tile size
NLT = L // LT                # 8 main l-tiles
LC = 32                      # correction l-chunk size
NLC = L // LC                # 16 correction chunks
NTB = T // 128               # 8 task blocks
NG = 2                       # psum groups per tb (4 l-tiles each)

_CACHE = {}


def _build_program():
    import concourse.tile as tile
    from concourse import bacc, mybir

    nc = bacc.Bacc("TRN2", target_bir_lowering=False, debug=False,
                   enable_asserts=False, num_devices=NCORES)
    f32 = mybir.dt.float32
    f16 = mybir.dt.float16
    bf16 = mybir.dt.bfloat16

    rh_in = nc.dram_tensor("rh", [L, T], bf16, kind="ExternalInput").ap()
    rc_in = nc.dram_tensor("rc", [NLC, 128, T], bf16,
                           kind="ExternalInput").ap()
    ghm_in = nc.dram_tensor("ghm", [NLT, 128, ACORE * LT], bf16,
                            kind="ExternalInput").ap()
    gc_in = nc.dram_tensor("gc", [NLC, 128, ACORE * LC], bf16,
                           kind="ExternalInput").ap()
    d_in = nc.dram_tensor("d", [128, NTB], f32, kind="ExternalInput").ap()
    # [g, t, a, l-within-group] so each partition's store is one 4KB
    # contiguous run; the host permutes back to [a, t, l].
    out = nc.dram_tensor("out", [NG, T, ACORE, 256], f16,
                         kind="ExternalOutput").ap()

    # Main-pass R chunk per l-tile: window j in [js, js+127], js = 0 if
    # lt==0 else 64*(lt-1).  Even-aligned windows (odd lt, and lt=0) come
    # from "A" chunks at j = 0,128,256,384; odd-aligned (even lt>=2) from
    # "B" chunks at j = 64,192,320.
    chunk_specs = [("A0", 0), ("A1", 128), ("A2", 256), ("A3", 384),
                   ("B0", 64), ("B1", 192), ("B2", 320)]
    lt_chunk = ["A0", "A0", "B0", "A1", "B1", "A2", "B2", "A3"]
    chunk_js = dict(chunk_specs)

    with tile.TileContext(nc) as tc:
        with tc.tile_pool(name="consts", bufs=1) as consts, \
             tc.tile_pool(name="outp", bufs=4) as outp, \
             tc.tile_pool(name="ps", bufs=2, space="PSUM") as psp:

            dsc = consts.tile([128, NTB], f32, tag="dsc")
            nc.sync.dma_start(dsc[:], d_in[:])

            rt, rc, ghm, gc = {}, {}, {}, {}

            def load_rh(name):
                js = chunk_js[name]
                t_ = consts.tile([128, T], bf16, tag=f"rh{name}")
                nc.sync.dma_start(t_[:], rh_in[js:js + 128, :])
                rt[name] = t_

            def load_rc(lc):
                t_ = consts.tile([128, T], bf16, tag=f"rc{lc}")
                nc.sync.dma_start(t_[:], rc_in[lc])
                rc[lc] = t_

            def load_ghm(lt):
                t_ = consts.tile([128, ACORE * LT], bf16, tag=f"ghm{lt}")
                nc.sync.dma_start(t_[:], ghm_in[lt])
                ghm[lt] = t_

            def load_gc(lc):
                t_ = consts.tile([128, ACORE * LC], bf16, tag=f"gc{lc}")
                nc.sync.dma_start(t_[:], gc_in[lc])
                gc[lc] = t_

            # emit loads in compute consumption order
            for name, lts in [("A0", [0, 1]), ("B0", [2]), ("A1", [3]),
                              ("B1", [4]), ("A2", [5]), ("B2", [6]),
                              ("A3", [7])]:
                load_rh(name)
                for lt in lts:
                    load_ghm(lt)
                    for lh in range(2):
                        lc = 2 * lt + lh
                        load_gc(lc)
                        load_rc(lc)

            for g in range(NG):
                for tb in range(NTB):
                    ps = psp.tile([128, 4 * 512], f32, tag="ps")
                    for sub in range(4):
                        lt = g * 4 + sub
                        pmain = ps[:, sub * 512:(sub + 1) * 512]
                        nc.tensor.matmul(
                            pmain,
                            lhsT=rt[lt_chunk[lt]][:, tb * 128:(tb + 1) * 128],
                            rhs=ghm[lt][:],
                            start=True, stop=False, skip_group_check=True)
                        for lh in range(2):
                            lc = 2 * lt + lh
                            pcorr = ps[:, sub * 512 + lh * 256:
                                       sub * 512 + (lh + 1) * 256]
                            nc.tensor.matmul(
                                pcorr,
                                lhsT=rc[lc][:, tb * 128:(tb + 1) * 128],
                                rhs=gc[lc][:],
                                start=False, stop=True,
                                skip_group_check=True)
                    # psum free layout: s*512 + lh*256 + a*32 + ll
                    # osb free layout:  a*256 + s*64 + lh*32 + ll
                    osb = outp.tile([128, ACORE * 256], f16, tag="osb")
                    nc.scalar.activation(
                        osb[:].rearrange("p (a s lh ll) -> p s lh a ll",
                                         a=ACORE, s=4, lh=2),
                        ps[:].rearrange("p (s lh a ll) -> p s lh a ll",
                                        s=4, lh=2, a=ACORE),
                        mybir.ActivationFunctionType.Tanh,
                        scale=dsc[:, tb:tb + 1])
                    nc.sync.dma_start(
                        out[g, tb * 128:(tb + 1) * 128],
                        osb[:].rearrange("p (a l) -> p a l", a=ACORE))

    nc.compile()
    return nc


def _host_chain(lx, task_matrix, task_difficulty, alg_efficiency,
                alg_memory, alg_experience_boost):
    """Exact (f64) scalar feedback chain + banded coefficient tensors."""
    import ml_dtypes
    bf = ml_dtypes.bfloat16

    lx = np.asarray(lx).astype(np.int64)
    TM = np.asarray(task_matrix, dtype=np.float64)
    diff = np.asarray(task_difficulty, dtype=np.float64)
    eff = np.asarray(alg_efficiency, dtype=np.float64)
    mem = np.asarray(alg_memory, dtype=np.float64)
    boost = np.asarray(alg_experience_boost, dtype=np.float64)

    R = TM[lx]                     # [L, T]
    TM2 = R[:, lx]                 # [L, L]
    dlx = diff[lx]                 # [L]

    resS = np.zeros((A, L))
    c = np.empty((A, L))
    for l in range(L):
        s_l = 2.0 / (1.0 + np.exp(-resS[:, l] / dlx[l])) - 1.0
        c[:, l] = eff + s_l * boost
        resS = resS * mem[:, None] + c[:, l][:, None] * TM2[l][None, :]

    Rf = R.astype(np.float32)
    Rh = Rf.astype(bf)
    Rl = (Rf - Rh.astype(np.float32)).astype(bf)
    Rhp = np.zeros((L + 64, T), dtype=bf)    # index shifted by +32
    Rlp = np.zeros((L + 64, T), dtype=bf)
    Rhp[32:32 + L] = Rh
    Rlp[32:32 + L] = Rl
    # Rc[lc]: partitions 0-63 = Rl[js_c .. js_c+63], 64-127 = Rh[same],
    # js_c = 32*lc - 32 (j<0 zero-padded)
    Rc = np.zeros((NLC, 128, T), dtype=bf)
    for lc in range(NLC):
        Rc[lc, :64] = Rlp[32 * lc:32 * lc + 64]
        Rc[lc, 64:] = Rhp[32 * lc:32 * lc + 64]

    def g_entries(a_mem, a_c, jw, lw):
        # [len(jw), len(lw)] = mem^(l-j) * c[j] for 0 <= l-j, else 0
        lmj = lw[None, :] - jw[:, None]
        valid = (lmj >= 0) & (jw[:, None] >= 0)
        pw = np.where(valid, a_mem ** np.maximum(lmj, 0), 0.0)
        return pw * np.where(jw >= 0, a_c[np.maximum(jw, 0)], 0.0)[:, None]

    # main G (hi part): per l-tile, window js(lt), free = lh*256+a*32+ll
    Ghm = np.zeros((A, NLT, 128, LT), dtype=np.float64)
    for lt in range(NLT):
        js = 0 if lt == 0 else 64 * (lt - 1)
        jw = np.arange(js, js + 128)
        lw = np.arange(64 * lt, 64 * lt + 64)
        for a in range(A):
            Ghm[a, lt] = g_entries(mem[a], c[a], jw, lw)
    Ghm_h = Ghm.astype(np.float32).astype(bf)           # hi part
    Ghm_l64 = Ghm - Ghm_h.astype(np.float64)            # lo residual

    # correction G: per l-chunk, window js_c = 32*lc-32;
    # partitions 0-63: Gh (pairs with Rl), 64-127: Gl (pairs with Rh)
    Gc = np.zeros((A, NLC, 128, LC), dtype=bf)
    for lc in range(NLC):
        js_c = 32 * lc - 32
        jw = np.arange(js_c, js_c + 64)
        lw = np.arange(32 * lc, 32 * lc + 32)
        for a in range(A):
            gh_w = g_entries(mem[a], c[a], jw, lw).astype(np.float32)
            Gc[a, lc, :64] = gh_w.astype(bf)
            # lo residual of G on the window
            gl_w = gh_w - Gc[a, lc, :64].astype(np.float32)
            Gc[a, lc, 64:] = gl_w.astype(bf)

    # pack per core
    def pack(Gx, width):
        # Gx: [A, n, 128, width] -> per core [n, 128, ACORE*width]
        packs = []
        for core in range(NCORES):
            blk = Gx[core * ACORE:(core + 1) * ACORE]
            packs.append(np.ascontiguousarray(
                blk.transpose(1, 2, 0, 3).reshape(-1, 128, ACORE * width)))
        return packs

    # main-G free layout must be lh*256 + a*32 + ll -> reshape LT=64 into
    # (lh 2, ll 32) and order (lh, a, ll)
    Ghm_b = Ghm_h.reshape(A, NLT, 128, 2, 32)
    ghm_packs = []
    for core in range(NCORES):
        blk = Ghm_b[core * ACORE:(core + 1) * ACORE]  # [ACORE,NLT,128,2,32]
        ghm_packs.append(np.ascontiguousarray(
            blk.transpose(1, 2, 3, 0, 4).reshape(NLT, 128, ACORE * LT)))
    gc_packs = pack(Gc, LC)

    dsc = np.ascontiguousarray(
        (1.0 / (2.0 * diff)).reshape(NTB, 128).T).astype(np.float32)
    return Rh, Rc, ghm_packs, gc_packs, dsc


def kernel(lx, task_matrix, task_difficulty, alg_efficiency, alg_memory,
           alg_experience_boost):
    from concourse.bass_utils import run_bass_kernel_spmd

    Rh, Rc, ghm_packs, gc_packs, dsc = _host_chain(
        lx, task_matrix, task_difficulty, alg_efficiency, alg_memory,
        alg_experience_boost)

    if "nc" not in _CACHE:
        _CACHE["nc"] = _build_program()
    nc = _CACHE["nc"]

    in_maps = [{"rh": Rh, "rc": Rc, "ghm": ghm_packs[c], "gc": gc_packs[c],
                "d": dsc} for c in range(NCORES)]
    res = run_bass_kernel_spmd(nc, in_maps, core_ids=list(range(NCORES)),
                               trace=False)
    out = np.empty((A, T, L + 1), dtype=np.float32)
    out[:, :, 0] = 0.0
    for c in range(NCORES):
        dev = res.results[c]["out"]          # [NG, T, ACORE, 256] f16
        out[c * ACORE:(c + 1) * ACORE, :, 1:] = (
            dev.transpose(2, 1, 0, 3).reshape(ACORE, T, L).astype(np.float32))
    return out
